# revision 47
# baseline (speedup 1.0000x reference)
"""Trainium2 Bass kernel: 2-layer CompGATv3 encoder + ConvE decoder (KG link scoring).

Sharding (8 NeuronCores, SPMD, full inputs in / full output out):
- Node-parallel GNN: core c owns entity rows [c*6250, (c+1)*6250). Host sorts
  edges by destination and buckets them into 128-node blocks; each block's
  edge list is padded to whole 128-edge tiles (schedule = per-block max over
  cores so one program serves all cores).
- Host prep ships per-edge-slot data so the device never runs indirect
  gathers for layer 1: composed messages (ent_emb[src] * rel_emb[et]) are
  pre-gathered AND pre-transposed (comp1T), the one-hot destination
  selector selT is prebuilt, and rel1/rel2 (tiny 500x200 weight folds) are
  computed on host so layer 2's relation rows (reL2) are pre-gathered too.
  Layer 2 keeps one indirect gather per tile (src rows of the
  device-computed ent1 table).
- The GATv2 destination term v = x @ [Wself | Wself@a] lives in SBUF
  (bf16); the per-edge dest row is produced by a one-hot matmul
  (selT.T @ v_block) instead of a DMA gather.
- Scatter-add into per-block PSUM accumulators via one-hot matmuls; segment
  softmax denominator rides along as a ones-column; epilogue divides,
  blends layer-1 attention (beta), adds bias, tanh, and DMA-transposes the
  block (SBUF->SBUF) into a resident entity-transpose table used by the
  next v-phase and the decoder score matmul.
- Collectives: AllGather of ent1 (layer boundary), small AllReduce of the
  decoder head rows (each core contributes its owned rows; no ent2
  AllGather needed), AllReduce of the projected z.
- Decoder: conv lowered to a sparse matrix; the rrep half of the conv is
  host-folded (rel2[r] is host-known); output-column-sharded over cores
  with a partial-z AllReduce; score matmul uses the resident e2T slice so
  the [B, n_ent] output is column-sharded.
"""

import math
import numpy as np
import ml_dtypes

import concourse.bacc as bacc
import concourse.bass as bass
import concourse.mybir as mybir
import concourse.tile as tile
import concourse.bass_utils as bass_utils
from concourse.bass import IndirectOffsetOnAxis
from concourse.masks import make_identity

F32 = mybir.dt.float32
BF16 = mybir.dt.bfloat16
I32 = mybir.dt.int32
AF = mybir.ActivationFunctionType
OP = mybir.AluOpType
BF16_NP = ml_dtypes.bfloat16

FULL_CFG = dict(n_ent=50000, n_rel=500, d=200, b=256, ncores=8,
                ent_h=10, ent_w=20, fc=32, fs=3)

BETA = 0.5
BN_EPS = 1e-5
LRELU_SLOPE = 0.2
SOFTMAX_EPS = 1e-16
PAD_COL = 999.0
OOB_SENTINEL = 1 << 20


# ---------------------------------------------------------------- host prep

def _ceil_div(a, b):
    return -(-a // b)


def _preprocess(inputs, cfg):
    ncores = cfg["ncores"]
    n_ent, n_rel, d, b = cfg["n_ent"], cfg["n_rel"], cfg["d"], cfg["b"]
    npc = n_ent // ncores
    nblk = _ceil_div(npc, 128)
    npad = nblk * 128

    src = np.asarray(inputs["edge_index"][0], np.int64)
    dst = np.asarray(inputs["edge_index"][1], np.int64)
    et = np.asarray(inputs["edge_type"], np.int64)

    # ---- balanced entity->slot permutation: pack nodes into (core, block)
    # bins of <=128 nodes aiming for <=512 in-edges per bin, so nearly every
    # block needs only ceil(512/128)=4 edge tiles (T ~= 4*nblk instead of the
    # ~5*nblk an unbalanced split needs). Scores are un-permuted on the host.
    import heapq
    deg = np.bincount(dst, minlength=n_ent).astype(np.int64)
    order = np.argsort(-deg, kind="stable")
    core_nodes = [[] for _ in range(ncores)]
    # LPT nodes -> cores (node cap npad per core)
    h1 = [(0, c) for c in range(ncores)]
    heapq.heapify(h1)
    core_load = [0] * ncores
    for n in order:
        while True:
            load, c = heapq.heappop(h1)
            if len(core_nodes[c]) < npad:
                break
        core_nodes[c].append(n)
        core_load[c] = load + int(deg[n])
        if len(core_nodes[c]) < npad:
            heapq.heappush(h1, (core_load[c], c))
    slot_of = np.empty(n_ent, np.int64)
    cap = 4 * 128  # per-bin edge budget for a 4-tile block
    for c in range(ncores):
        nodes = core_nodes[c]
        h2 = [(0, bi) for bi in range(nblk)]
        heapq.heapify(h2)
        bins = [[] for _ in range(nblk)]
        bl = [0] * nblk
        for n in nodes:  # already degree-desc within core
            while True:
                load, bi = heapq.heappop(h2)
                if len(bins[bi]) < 128:
                    break
            bins[bi].append(n)
            bl[bi] = load + int(deg[n])
            if len(bins[bi]) < 128:
                heapq.heappush(h2, (bl[bi], bi))
        # swap-repair: push overloaded bins under cap using underloaded ones
        for _ in range(4 * nblk):
            bo = int(np.argmax(bl))
            if bl[bo] <= cap:
                break
            done = False
            for bu in np.argsort(bl):
                bu = int(bu)
                if bl[bu] >= cap or bu == bo:
                    continue
                need = bl[bo] - cap
                room = cap - bl[bu]
                dbo = np.array([deg[n] for n in bins[bo]])
                dbu = np.array([deg[n] for n in bins[bu]])
                diff = dbo[:, None] - dbu[None, :]
                ok = (diff > 0) & (diff <= room)
                if not ok.any():
                    continue
                gain = np.where(ok, np.minimum(diff, need), -1)
                i, jj = np.unravel_index(np.argmax(gain), gain.shape)
                ni, nj = bins[bo][i], bins[bu][jj]
                bins[bo][i], bins[bu][jj] = nj, ni
                delta = int(deg[ni] - deg[nj])
                bl[bo] -= delta
                bl[bu] += delta
                done = True
                break
            if not done:
                break
        ordb = np.argsort(-np.asarray(bl), kind="stable")
        for newb, oldb in enumerate(ordb):
            for i, n in enumerate(bins[int(oldb)]):
                slot_of[n] = c * npad + newb * 128 + i

    # chunked-AllGather table layout (chunk-major, rank-major within chunk)
    nag = 4
    chunk_of_blk = np.minimum(np.arange(nblk) // _ceil_div(nblk, nag), nag - 1)
    ag_rows = np.array([int((chunk_of_blk == k).sum()) * 128 for k in range(nag)])
    ag_off = np.zeros(nag, np.int64)     # first loc row of chunk
    ag_off[1:] = np.cumsum(ag_rows)[:-1]
    ag_tb = np.zeros(nag, np.int64)      # table base row of chunk
    ag_tb[1:] = np.cumsum(ag_rows * ncores)[:-1]
    sall = np.arange(ncores * npad)
    score_ = sall // npad
    loc_ = sall % npad
    k_ = chunk_of_blk[loc_ // 128]
    trow = ag_tb[k_] + score_ * ag_rows[k_] + (loc_ - ag_off[k_])

    dslot = slot_of[dst]
    core_of = dslot // npad
    cnts = np.zeros((ncores, nblk), np.int64)
    percore = []
    for c in range(ncores):
        m = core_of == c
        s_c, t_c = src[m], et[m]
        loc = (dslot[m] - c * npad).astype(np.int64)
        o = np.argsort(loc, kind="stable")
        s_c, t_c, loc = s_c[o], t_c[o], loc[o]
        blk = loc // 128
        cnts[c] = np.bincount(blk, minlength=nblk)
        percore.append((s_c, t_c, loc, blk))

    tpb = np.maximum(1, _ceil_div(cnts.max(axis=0), 128)).astype(np.int64)
    T = int(tpb.sum())
    tile_blk = np.repeat(np.arange(nblk), tpb)
    tstart = np.zeros(nblk, np.int64)
    tstart[1:] = np.cumsum(tpb)[:-1]

    f32 = lambda x: np.ascontiguousarray(np.asarray(x, np.float32))
    bf = lambda x: np.ascontiguousarray(np.asarray(x, np.float32).astype(BF16_NP))

    ent_emb = f32(inputs["ent_emb"])
    rel_emb = f32(inputs["rel_emb"])
    rel1 = rel_emb @ f32(inputs["Wrel1"])
    rel2 = rel1 @ f32(inputs["Wrel2"])

    # one packed DMA per edge tile per layer:
    # l1pack tile cols [0:128]=comp1T_hi, [128:256] rows0:72=comp1T_lo,
    #   [256:384]=selT;  l2pack tile cols [0:200]=re rows, [200:328]=selT
    srcT = np.zeros((ncores, 128, T), np.int32)
    colT = np.full((ncores, 128, T), PAD_COL, np.float32)
    l1pack = np.zeros((ncores, 128, T * 384), BF16_NP)
    l2pack = np.zeros((ncores, 128, T * 328), BF16_NP)
    for c in range(ncores):
        s_c, t_c, loc, blk = percore[c]
        off = np.zeros(nblk, np.int64)
        off[1:] = np.cumsum(cnts[c])[:-1]
        wb = np.arange(len(s_c)) - off[blk]          # index within block
        slot = tstart[blk] * 128 + wb                # flat slot in [T*128]
        fs_ = np.zeros(T * 128, np.int32)
        fc_ = np.full(T * 128, PAD_COL, np.float32)
        fs_[slot] = trow[slot_of[s_c]]
        fc_[slot] = (loc % 128).astype(np.float32)
        srcT[c] = fs_.reshape(T, 128).T
        colT[c] = fc_.reshape(T, 128).T
        comp1 = np.zeros((T * 128, d), np.float32)
        comp1[slot] = ent_emb[s_c] * rel_emb[t_c]
        c1t = comp1.T.astype(BF16_NP)                # [d, T*128]
        re2 = np.zeros((T * 128, d), np.float32)
        re2[slot] = rel1[t_c]
        re2 = re2.astype(BF16_NP)
        st = np.zeros((128, T * 128), np.float32)
        st[(loc % 128).astype(np.int64), slot] = 1.0
        st = st.astype(BF16_NP)
        p1 = l1pack[c].reshape(128, T, 384)
        p1[:, :, 0:128] = c1t[:128].reshape(128, T, 128)
        p1[:d - 128, :, 128:256] = c1t[128:].reshape(d - 128, T, 128)
        p1[:, :, 256:384] = st.reshape(128, T, 128)
        p2 = l2pack[c].reshape(128, T, 328)
        p2[:, :, 0:d] = re2.reshape(T, 128, d).transpose(1, 0, 2)
        p2[:, :, d:328] = st.reshape(128, T, 128)

    def aug(w, a):
        # [d, d+1]: last column is w @ a (linear part of the attention logit)
        w = f32(w)
        return np.concatenate([w, (w @ f32(a))[:, None]], axis=1)

    entT = []
    for c in range(ncores):
        m = (slot_of // npad) == c
        locs = slot_of[m] - c * npad
        sl = np.zeros((d, npad), np.float32)
        sl[:, locs] = ent_emb[m].T
        entT.append(bf(sl))

    # ---- decoder prep
    ent_h, ent_w, fc, fs_k = cfg["ent_h"], cfg["ent_w"], cfg["fc"], cfg["fs"]
    hh, ww = 2 * ent_h, ent_w                 # image dims (20, 20)
    oh, ow = hh - fs_k + 1, ww - fs_k + 1     # conv output (18, 18)
    num_in = fc * oh * ow
    npix = hh * ww                            # 400
    conv_w = f32(inputs["conv_w"])            # [fc, 1, fs, fs]
    g0p = float(np.asarray(inputs["bn0_g"], np.float32)[0] / math.sqrt(1.0 + BN_EPS))
    b0 = float(np.asarray(inputs["bn0_b"], np.float32)[0])
    g1p = f32(inputs["bn1_g"]) / math.sqrt(1.0 + BN_EPS)
    b1v = f32(inputs["bn1_b"])
    gpp = f32(inputs["bnp_g"]) / math.sqrt(1.0 + BN_EPS)
    bpv = f32(inputs["bnp_b"])
    prelu1 = float(np.asarray(inputs["prelu1"], np.float32).ravel()[0])
    prelu2 = float(np.asarray(inputs["prelu2"], np.float32).ravel()[0])

    big_w = np.zeros((npix, num_in), np.float32)
    oy, ox = np.meshgrid(np.arange(oh), np.arange(ow), indexing="ij")
    for oc in range(fc):
        for dy in range(fs_k):
            for dx in range(fs_k):
                pix = (oy + dy) * ww + (ox + dx)
                out_i = oc * (oh * ow) + oy * ow + ox
                big_w[pix, out_i] = conv_w[oc, 0, dy, dx] * g0p
    # pixel reorder: [head dims 0..d-1, tail dims 0..d-1] (orig interleaved 2d, 2d+1)
    perm = np.concatenate([np.arange(d) * 2, np.arange(d) * 2 + 1])
    big_w = big_w[perm]

    ridx = np.asarray(inputs["r"], np.int64)
    rrep = rel2[ridx]                         # [B, d] host-known

    ocpc = num_in // ncores          # out-columns per core
    occ = fc // ncores               # conv channels per core
    sumw = conv_w.reshape(fc, -1).sum(1)
    nchunk = _ceil_div(ocpc, 128)
    acol = np.zeros((ncores, nchunk * 128, 1), np.float32)
    ccol = np.zeros((ncores, nchunk * 128, 1), np.float32)
    rrep_convT = np.zeros((ncores, nchunk * 128, b), BF16_NP)
    for c in range(ncores):
        ocs = np.arange(ocpc) // (oh * ow) + c * occ
        acol[c, :ocpc, 0] = g1p[ocs]
        ccol[c, :ocpc, 0] = g1p[ocs] * b0 * sumw[ocs] + b1v[ocs]
        rc = rrep @ big_w[d:, c * ocpc:(c + 1) * ocpc]   # [B, ocpc]
        rrep_convT[c, :ocpc] = rc.T.astype(BF16_NP)

    acol_a = acol * prelu1           # scale/bias for the linear branch of prelu
    ccol_a = ccol * prelu1

    pw = f32(inputs["proj_w"]) * gpp[None, :]
    pb = f32(inputs["proj_b"]) * gpp + bpv
    pwc = np.zeros((ncores, ocpc + 1, d), np.float32)
    for c in range(ncores):
        pwc[c, :ocpc] = pw[c * ocpc:(c + 1) * ocpc]
    pwc[0, ocpc] = pb                      # bias row only on core 0 (AllReduce sums)

    bias_ent = f32(inputs["bias_ent"])
    bias_sl = np.zeros((ncores, 1, npad), np.float32)
    for c in range(ncores):
        m = (slot_of // npad) == c
        bias_sl[c, 0, slot_of[m] - c * npad] = bias_ent[m]

    hidx = np.asarray(inputs["h"], np.int64)
    hslot = slot_of[hidx]
    bb = b // 128                           # batch chunks (2)
    hloc = np.full((ncores, 128, bb), OOB_SENTINEL, np.int32)
    for c in range(ncores):
        own = (hslot // npad) == c
        hl = np.where(own, hslot - c * npad, OOB_SENTINEL).astype(np.int32)
        hloc[c] = hl.reshape(bb, 128).T

    common = {
        "W1": bf(aug(inputs["W1"], inputs["a1"])),
        "Ws1": bf(aug(inputs["Wself1"], inputs["a1"])),
        "W2": bf(aug(inputs["W2"], inputs["a2"])),
        "Ws2": bf(aug(inputs["Wself2"], inputs["a2"])),
        "A1m": f32(np.broadcast_to(np.asarray(inputs["a1"], np.float32), (128, d))),
        "A2m": f32(np.broadcast_to(np.asarray(inputs["a2"], np.float32), (128, d))),
        "B1m": f32(np.broadcast_to(np.asarray(inputs["b1"], np.float32), (128, d))),
        "B2m": f32(np.broadcast_to(np.asarray(inputs["b2"], np.float32), (128, d))),
    }
    per_core = []
    for c in range(ncores):
        per_core.append({
            "srcT": srcT[c], "colT": colT[c],
            "l1pack": l1pack[c], "l2pack": l2pack[c],
            "entT_hi": entT[c][:128], "entT_lo": entT[c][128:d],
            "bigW": bf(big_w[:d, c * ocpc:(c + 1) * ocpc]),
            "rrep_convT": rrep_convT[c],
            "acol": acol[c], "ccol": ccol[c],
            "acol_a": acol_a[c], "ccol_a": ccol_a[c],
            "pwc": bf(pwc[c]),
            "bias_sl": bf(bias_sl[c]),
            "hloc": hloc[c],
        })

    sched = dict(T=T, tpb=tpb, tile_blk=tile_blk, tstart=tstart,
                 npc=npad, nblk=nblk, npad=npad, nchunk=nchunk, ocpc=ocpc,
                 prelu1=prelu1, prelu2=prelu2, npix=npix, bb=bb,
                 slot_of=slot_of, nag=nag, chunk_of_blk=chunk_of_blk,
                 ag_rows=ag_rows, ag_off=ag_off, ag_tb=ag_tb)
    return common, per_core, sched


# ---------------------------------------------------------------- device code

def _dchunks(d):
    out = []
    s = 0
    while s < d:
        out.append((s, min(s + 128, d)))
        s += 128
    return out


def _emit_v_phase(nc, pool, ps_pool, xT_hi, xT_lo, w_dram, v_sb, nblk, d):
    """v_sb[:, blk] = x[n,:] @ [Wself | Wself@a] for owned nodes (bf16 SBUF)."""
    dch = _dchunks(d)
    w_sb = []
    for (s, e) in dch:
        wt = pool.tile([e - s, d + 1], BF16, tag=f"vw{s}")
        nc.sync.dma_start(wt[:], w_dram[s:e, :])
        w_sb.append(wt)
    for bkt in range(nblk):
        vps = ps_pool.tile([128, d + 1], F32, tag="ups")
        nc.tensor.matmul(vps[:], lhsT=xT_hi[0:128, bkt * 128:(bkt + 1) * 128],
                         rhs=w_sb[0][:], start=True, stop=False)
        nc.tensor.matmul(vps[:], lhsT=xT_lo[0:d - 128, bkt * 128:(bkt + 1) * 128],
                         rhs=w_sb[1][:], start=False, stop=True)
        nc.vector.tensor_copy(v_sb[:, bkt * (d + 1):(bkt + 1) * (d + 1)], vps[:])


def _emit_edge_layer(nc, layer, cst, epool, ps_u, ps_vd, ps_acc, sched, di,
                     x_tab, w_sb, amat, bmat, srcT_sb, colT_sb, iota_f32,
                     v_sb, uaug3, ident_bf, ebf3, ex_store, rd1_store,
                     sf1_store, ent_sl, eT_hi, eT_lo, d, ag=None):
    """One CompGAT layer over all edge tiles + per-block epilogues.

    The destination self-term rides in the message PSUM (third K-chunk via
    the one-hot selT), so the accumulated scatter output is sum_e w_e*z_e;
    the epilogue subtracts v_n * (normalized weight sum) to recover
    sum_e w_e*msg_e exactly (also correct for isolated nodes).
    """
    T, tpb, tile_blk, tstart = sched["T"], sched["tpb"], sched["tile_blk"], sched["tstart"]
    dlo = d - 128
    pend = []
    delay = 1 if layer == 1 else 0
    state = {}

    def _flush(item):
        _emit_scatter_epilogue(nc, layer, epool, ps_acc, sched, state, bmat,
                               colT_sb, v_sb, ebf3, rd1_store, sf1_store,
                               ent_sl, eT_hi, eT_lo, d, ag, item)

    for t in range(T):
        bkt = int(tile_blk[t])
        j = t - int(tstart[bkt])
        last = j == int(tpb[bkt]) - 1

        if layer == 1:
            pk = epool.tile([128, 384], BF16, tag="pk")
            nc.sync.dma_start(pk[:], di["l1pack"][:, t * 384:(t + 1) * 384])
            lhsA, lhsB = pk[:, 0:128], pk[0:dlo, 128:256]
            sT = pk[:, 256:384]
        else:
            pk = epool.tile([128, 328], BF16, tag="pk")
            nc.sync.dma_start(pk[:], di["l2pack"][:, t * 328:(t + 1) * 328])
            sT = pk[:, d:328]
            xs = epool.tile([128, d], BF16, tag="xs")
            nc.gpsimd.indirect_dma_start(
                out=xs[:], out_offset=None, in_=x_tab[:, :],
                in_offset=IndirectOffsetOnAxis(ap=srcT_sb[:, t:t + 1], axis=0))
            cm = epool.tile([128, d], BF16, tag="cm")
            nc.vector.tensor_tensor(out=cm[:], in0=xs[:], in1=pk[:, 0:d], op=OP.mult)
            trA = ps_vd.tile([128, 128], BF16, tag="vd")
            nc.tensor.transpose(out=trA[:], in_=cm[:, 0:128], identity=ident_bf[:])
            ctA = epool.tile([128, 128], BF16, tag="ctA")
            nc.vector.tensor_copy(ctA[:], trA[:])
            trB = ps_vd.tile([128, 128], BF16, tag="vd")
            nc.tensor.transpose(out=trB[0:dlo, :], in_=cm[:, 128:d], identity=ident_bf[:])
            ctB = epool.tile([dlo, 128], BF16, tag="ctB")
            nc.scalar.copy(ctB[:], trB[0:dlo, :])
            lhsA, lhsB = ctA[:, :], ctB[:, :]

        # z = (x_src*rel) @ Waug + v_dst, all in one PSUM accumulation
        ups = ps_u.tile([128, d + 1], F32, tag="ups")
        nc.tensor.matmul(ups[:], lhsT=lhsA, rhs=w_sb[0][:], start=True, stop=False)
        nc.tensor.matmul(ups[:], lhsT=lhsB, rhs=w_sb[1][:], start=False, stop=False)
        nc.tensor.matmul(ups[:], lhsT=sT,
                         rhs=v_sb[:, bkt * (d + 1):(bkt + 1) * (d + 1)],
                         start=False, stop=True)
        ua = uaug3[t % 3]
        if layer == 1:
            nc.scalar.copy(ua[:, 0:d + 1], ups[:])
        # lrelu(z)@a = c1*(z@a) + c2*(|z|@a), c1=(1+s)/2, c2=(1-s)/2
        abz = epool.tile([128, d], F32, tag="abz")
        nc.scalar.activation(abz[:], ups[:, 0:d], AF.Abs)
        e0 = epool.tile([128, 1], F32, tag="e0")
        nc.vector.tensor_scalar(out=e0[:], in0=ups[:, d:d + 1],
                                scalar1=(1.0 + LRELU_SLOPE) / 2.0,
                                scalar2=None, op0=OP.mult)
        ttro = epool.tile([128, d], F32, tag="ttro")
        e_sb = epool.tile([128, 1], F32, tag="esb")
        nc.vector.scalar_tensor_tensor(out=ttro[:], in0=abz[:], scalar=1.0,
                                       in1=amat[:], op0=OP.mult, op1=OP.mult,
                                       accum_out=e_sb[:])
        if layer == 1:
            ex_ap = ex_store[:, t:t + 1]
        else:
            ex_t = epool.tile([128, 1], F32, tag="ex2")
            ex_ap = ex_t[:, :]
        nc.scalar.activation(ex_ap, e_sb[:], AF.Exp,
                             scale=(1.0 - LRELU_SLOPE) / 2.0, bias=e0[:, 0:1])
        if layer == 1:
            # one-hot scatter, lhsT carries the ex1 weight
            sex = epool.tile([128, 128], BF16, tag="sex")
            nc.vector.tensor_scalar(out=sex[:], in0=iota_f32[:],
                                    scalar1=colT_sb[:, t:t + 1], scalar2=ex_ap,
                                    op0=OP.is_equal, op1=OP.mult)
            rhs_ap = ua[:, 0:d + 2]
        else:
            # single scatter, plain one-hot lhsT; rhs carries the weights:
            # cols 0:d+1 = ex2*z (scale folded into the PSUM cast), col d+1
            # = ex2, cols d+2: = ex1*z (built on DVE straight from PSUM)
            sex = epool.tile([128, 128], BF16, tag="sex")
            nc.vector.tensor_scalar(out=sex[:], in0=iota_f32[:],
                                    scalar1=colT_sb[:, t:t + 1], scalar2=None,
                                    op0=OP.is_equal)
            nc.scalar.activation(ua[:, 0:d + 1], ups[:, 0:d + 1], AF.Identity,
                                 scale=ex_ap)
            nc.vector.tensor_copy(ua[:, d + 1:d + 2], ex_ap)
            nc.vector.tensor_scalar(out=ua[:, d + 2:2 * d + 2], in0=ups[:, 0:d],
                                    scalar1=ex_store[:, t:t + 1], scalar2=None,
                                    op0=OP.mult)
            rhs_ap = ua[:, :]
        # scatter + epilogue are deferred one tile so the tensor engine can
        # start the next tile's matmuls while this tile's logit chain runs
        pend.append((sex, rhs_ap, bkt, j, last))
        if len(pend) == delay + 1:
            _flush(pend.pop(0))
    while pend:
        _flush(pend.pop(0))


def _emit_scatter_epilogue(nc, layer, epool, ps_acc, sched, state, bmat,
                           colT_sb, v_sb, ebf3, rd1_store, sf1_store,
                           ent_sl, eT_hi, eT_lo, d, ag, item):
    sex, rhs_ap, bkt, j, last = item
    if j == 0:
        w = (d + 2) if layer == 1 else (2 * d + 2)
        state["acc"] = ps_acc.tile([128, w], F32, tag="acc", name="acc")
    acc = state["acc"]
    nc.tensor.matmul(acc[:], lhsT=sex[:], rhs=rhs_ap,
                     start=(j == 0), stop=last)
    if last:
            vblk = v_sb[:, bkt * (d + 1):bkt * (d + 1) + d]
            den_eps = epool.tile([128, 1], F32, tag="deneps")
            nc.vector.tensor_scalar(out=den_eps[:], in0=acc[:, d + 1:d + 2],
                                    scalar1=SOFTMAX_EPS, scalar2=None, op0=OP.add)
            if layer == 1:
                rd_ap = rd1_store[:, bkt:bkt + 1]
                nc.vector.reciprocal(rd_ap, den_eps[:])
                # sf1 = den1*rd1 (1 if node has edges else 0)
                nc.vector.tensor_scalar(out=sf1_store[:, bkt:bkt + 1],
                                        in0=acc[:, d + 1:d + 2], scalar1=rd_ap,
                                        scalar2=None, op0=OP.mult)
                t1 = epool.tile([128, d], F32, tag="ep_t1")
                nc.vector.tensor_scalar(out=t1[:], in0=acc[:, 0:d], scalar1=rd_ap,
                                        scalar2=None, op0=OP.mult)
                tcor = epool.tile([128, d], F32, tag="ep_tc")
                nc.vector.tensor_scalar(out=tcor[:], in0=vblk,
                                        scalar1=sf1_store[:, bkt:bkt + 1],
                                        scalar2=None, op0=OP.mult)
            else:
                rd2 = epool.tile([128, 1], F32, tag="rd2")
                nc.vector.reciprocal(rd2[:], den_eps[:])
                sf2 = epool.tile([128, 1], F32, tag="sf2")
                nc.vector.tensor_scalar(out=sf2[:], in0=acc[:, d + 1:d + 2],
                                        scalar1=rd2[:, :], scalar2=1.0 - BETA,
                                        op0=OP.mult, op1=OP.mult)
                cf = epool.tile([128, 1], F32, tag="cf")
                nc.vector.tensor_scalar(out=cf[:], in0=sf1_store[:, bkt:bkt + 1],
                                        scalar1=BETA, scalar2=sf2[:, :],
                                        op0=OP.mult, op1=OP.add)
                tB = epool.tile([128, d], F32, tag="ep_tB")
                nc.vector.tensor_scalar(out=tB[:], in0=acc[:, 0:d], scalar1=rd2[:, :],
                                        scalar2=1.0 - BETA, op0=OP.mult, op1=OP.mult)
                tA = epool.tile([128, d], F32, tag="ep_tA")
                nc.vector.tensor_scalar(out=tA[:], in0=acc[:, d + 2:2 * d + 2],
                                        scalar1=rd1_store[:, bkt:bkt + 1],
                                        scalar2=BETA, op0=OP.mult, op1=OP.mult)
                t1 = epool.tile([128, d], F32, tag="ep_t1")
                nc.vector.tensor_tensor(out=t1[:], in0=tA[:], in1=tB[:], op=OP.add)
                tcor = epool.tile([128, d], F32, tag="ep_tc")
                nc.vector.tensor_scalar(out=tcor[:], in0=vblk,
                                        scalar1=cf[:, :], scalar2=None, op0=OP.mult)
            tsub = epool.tile([128, d], F32, tag="ep_ts")
            nc.vector.tensor_tensor(out=tsub[:], in0=t1[:], in1=tcor[:], op=OP.subtract)
            t2 = epool.tile([128, d], F32, tag="ep_t2")
            nc.vector.tensor_tensor(out=t2[:], in0=tsub[:], in1=bmat[:], op=OP.add)
            ebf = ebf3[bkt % 3]
            nc.scalar.activation(ebf[:, 0:d], t2[:], AF.Tanh)
            if isinstance(ent_sl, list):
                k = int(sched["chunk_of_blk"][bkt])
                row = bkt * 128 - int(sched["ag_off"][k])
                nc.sync.dma_start(ent_sl[k][row:row + 128, :], ebf[:, 0:d])
            else:
                nc.sync.dma_start(ent_sl[bkt * 128:(bkt + 1) * 128, :],
                                  ebf[:, 0:d])
            nc.sync.dma_start_transpose(
                eT_hi[:, bkt * 128:(bkt + 1) * 128], ebf[:, 0:128])
            nc.scalar.dma_start_transpose(
                eT_lo[:, bkt * 128:(bkt + 1) * 128], ebf[:, 128:256])
            if ag is not None and bkt in ag:
                ful, kk, t0, t1 = ag[bkt]
                nc.gpsimd.collective_compute(
                    "AllGather", OP.bypass,
                    replica_groups=[list(range(FULL_CFG["ncores"]))],
                    ins=[ent_sl[kk][:, :]], outs=[ful[t0:t1, :]])


def _emit_decoder(nc, cst, pool, ps_pool, sched, di, tensors, d, b, prelu1, prelu2):
    npad, nchunk, ocpc, bb = sched["npad"], sched["nchunk"], sched["ocpc"], sched["bb"]
    npc = sched["npc"]
    (ent2_sl, e2T_hi, e2T_lo, head_in, head_ar, z_in, z_ar, scores_out,
     ident_bf, ones_row) = tensors
    dch = _dchunks(d)
    dlo = d - 128

    hloc_sb = cst.tile([128, bb], I32, tag="hloc")
    nc.sync.dma_start(hloc_sb[:], di["hloc"][:, :])

    # each core contributes its owned head rows; AllReduce assembles [B, d]
    for bc in range(bb):
        hp = pool.tile([128, d], BF16, tag="dec_hp")
        nc.gpsimd.memset(hp[:], 0.0)
        nc.gpsimd.indirect_dma_start(
            out=hp[:], out_offset=None, in_=ent2_sl[:, :],
            in_offset=IndirectOffsetOnAxis(ap=hloc_sb[:, bc:bc + 1], axis=0),
            bounds_check=npad - 1, oob_is_err=False)
        nc.sync.dma_start(head_in[bc * 128:(bc + 1) * 128, :], hp[:])
    nc.gpsimd.collective_compute(
        "AllReduce", OP.add, replica_groups=[list(range(FULL_CFG["ncores"]))],
        ins=[head_in.ap()], outs=[head_ar.ap()])

    # head imgT K-chunks [d-chunks x b] via PE transposes
    imgT = []
    for (s, e) in dch:
        t = cst.tile([e - s, bb * 128], BF16, tag=f"imgTh{s}")
        imgT.append(t)
    for bc in range(bb):
        head = pool.tile([128, d], BF16, tag="dec_head")
        nc.sync.dma_start(head[:], head_ar[bc * 128:(bc + 1) * 128, :])
        for i, (s, e) in enumerate(dch):
            tp = ps_pool.tile([128, 128], BF16, tag="ups")
            nc.tensor.transpose(out=tp[0:e - s, 0:128], in_=head[:, s:e],
                                identity=ident_bf[:])
            nc.scalar.copy(imgT[i][:, bc * 128:(bc + 1) * 128], tp[0:e - s, 0:128])

    # conv via sparse matrix: head K-chunks on device, rrep half host-folded
    bw_sb = []
    for i, (s, e) in enumerate(dch):
        t = cst.tile([e - s, ocpc], BF16, tag=f"bw{i}")
        nc.sync.dma_start(t[:], di["bigW"][s:e, :])
        bw_sb.append(t)
    acol_sb = cst.tile([128, nchunk], F32, tag="acol")
    nc.sync.dma_start(acol_sb[:], di["acol"][:, :].rearrange("(c p) o -> p (c o)", p=128))
    ccol_sb = cst.tile([128, nchunk], F32, tag="ccol")
    nc.sync.dma_start(ccol_sb[:], di["ccol"][:, :].rearrange("(c p) o -> p (c o)", p=128))
    acola_sb = cst.tile([128, nchunk], F32, tag="acola")
    nc.sync.dma_start(acola_sb[:], di["acol_a"][:, :].rearrange("(c p) o -> p (c o)", p=128))
    ccola_sb = cst.tile([128, nchunk], F32, tag="ccola")
    nc.sync.dma_start(ccola_sb[:], di["ccol_a"][:, :].rearrange("(c p) o -> p (c o)", p=128))

    yT = []
    for ci in range(nchunk):
        cols = min(128, ocpc - ci * 128)
        yt = cst.tile([cols, bb * 128], BF16, tag=f"yT{ci}")
        cps = ps_pool.tile([128, bb * 128], F32, tag="ups")
        nc.tensor.matmul(cps[0:cols, :], lhsT=bw_sb[0][:, ci * 128:ci * 128 + cols],
                         rhs=imgT[0][:], start=True, stop=False)
        nc.tensor.matmul(cps[0:cols, :], lhsT=bw_sb[1][:, ci * 128:ci * 128 + cols],
                         rhs=imgT[1][:], start=False, stop=True)
        rT = pool.tile([128, bb * 128], BF16, tag="dec_rT")
        nc.sync.dma_start(rT[:], di["rrep_convT"][ci * 128:(ci + 1) * 128, :])
        tmp = pool.tile([128, bb * 128], F32, tag="dec_tmp")
        nc.vector.tensor_tensor(out=tmp[0:cols, :], in0=cps[0:cols, :],
                                in1=rT[0:cols, :], op=OP.add)
        # prelu(w) = a*w + (1-a)*relu(w), w = A*conv + C
        wlin = pool.tile([128, bb * 128], F32, tag="dec_wlin")
        nc.scalar.activation(wlin[0:cols, :], tmp[0:cols, :], AF.Identity,
                             scale=acola_sb[0:cols, ci:ci + 1],
                             bias=ccola_sb[0:cols, ci:ci + 1])
        wrel = pool.tile([128, bb * 128], F32, tag="dec_wrel")
        nc.scalar.activation(wrel[0:cols, :], tmp[0:cols, :], AF.Relu,
                             scale=acol_sb[0:cols, ci:ci + 1],
                             bias=ccol_sb[0:cols, ci:ci + 1])
        wrs = pool.tile([128, bb * 128], F32, tag="dec_wrs")
        nc.vector.tensor_scalar(out=wrs[0:cols, :], in0=wrel[0:cols, :],
                                scalar1=1.0 - prelu1, scalar2=None, op0=OP.mult)
        nc.vector.tensor_tensor(out=yt[0:cols, :], in0=wlin[0:cols, :],
                                in1=wrs[0:cols, :], op=OP.add)
        yT.append(yt)

    # proj: z[b, d] partial = sum_ci yT_ci.T @ pw_ci  + ones.T @ pb (core 0 only)
    pbrow = cst.tile([1, d], BF16, tag="pbrow")
    nc.sync.dma_start(pbrow[:], di["pwc"][ocpc:ocpc + 1, :])
    pw_sb = []
    for ci in range(nchunk):
        cols = min(128, ocpc - ci * 128)
        pwt = cst.tile([cols, d], BF16, tag=f"pw{ci}", name=f"pw{ci}")
        nc.sync.dma_start(pwt[:], di["pwc"][ci * 128:ci * 128 + cols, :])
        pw_sb.append(pwt)
    for bc in range(bb):
        zps = ps_pool.tile([128, d], F32, tag="ups")
        for ci in range(nchunk):
            nc.tensor.matmul(zps[:], lhsT=yT[ci][:, bc * 128:(bc + 1) * 128],
                             rhs=pw_sb[ci][:], start=(ci == 0), stop=False)
        nc.tensor.matmul(zps[:], lhsT=ones_row[0:1, bc * 128:(bc + 1) * 128],
                         rhs=pbrow[:], start=False, stop=True)
        zsb = pool.tile([128, d], F32, tag="dec_zsb")
        nc.vector.tensor_copy(zsb[:], zps[:])
        nc.sync.dma_start(z_in[bc * 128:(bc + 1) * 128, :], zsb[:])

    nc.gpsimd.collective_compute(
        "AllReduce", OP.add, replica_groups=[list(range(FULL_CFG["ncores"]))],
        ins=[z_in.ap()], outs=[z_ar.ap()])

    # prelu2 + transpose z2
    z2 = pool.tile([128, bb * d], F32, tag="z2")
    for bc in range(bb):
        nc.sync.dma_start(z2[:, bc * d:(bc + 1) * d], z_ar[bc * 128:(bc + 1) * 128, :])
    z2r = pool.tile([128, bb * d], F32, tag="z2r")
    nc.scalar.activation(z2r[:], z2[:], AF.Relu, scale=1.0 - prelu2)
    z2l = pool.tile([128, bb * d], F32, tag="z2l")
    nc.vector.tensor_scalar(out=z2l[:], in0=z2[:], scalar1=prelu2, scalar2=None,
                            op0=OP.mult)
    z2p = pool.tile([128, bb * d], BF16, tag="z2p")
    nc.vector.tensor_tensor(out=z2p[:], in0=z2l[:], in1=z2r[:], op=OP.add)
    z2T_hi = cst.tile([128, bb * 128], BF16, tag="z2T_hi")
    z2T_lo = cst.tile([dlo, bb * 128], BF16, tag="z2T_lo")
    for bc in range(bb):
        for i, (s, e) in enumerate(dch):
            tp = ps_pool.tile([128, 128], BF16, tag="ups")
            nc.tensor.transpose(out=tp[0:e - s, 0:128],
                                in_=z2p[:, bc * d + s:bc * d + e], identity=ident_bf[:])
            tgt = z2T_hi if i == 0 else z2T_lo
            nc.scalar.copy(tgt[0:e - s, bc * 128:(bc + 1) * 128], tp[0:e - s, 0:128])

    biasrow = cst.tile([1, npad], BF16, tag="biasrow")
    nc.sync.dma_start(biasrow[:], di["bias_sl"][0:1, :])

    for ns in range(0, npad, 512):
        ne = min(ns + 512, npad)
        for bc in range(bb):
            sps = ps_pool.tile([128, ne - ns], F32, tag="ups")
            nc.tensor.matmul(sps[:], lhsT=z2T_hi[:, bc * 128:(bc + 1) * 128],
                             rhs=e2T_hi[:, ns:ne], start=True, stop=False)
            nc.tensor.matmul(sps[:], lhsT=z2T_lo[:, bc * 128:(bc + 1) * 128],
                             rhs=e2T_lo[0:dlo, ns:ne], start=False, stop=False)
            nc.tensor.matmul(sps[:], lhsT=ones_row[0:1, bc * 128:(bc + 1) * 128],
                             rhs=biasrow[0:1, ns:ne], start=False, stop=True)
            ssb = pool.tile([128, ne - ns], F32, tag="dec_ssb")
            if bc % 2 == 0:
                nc.vector.tensor_copy(ssb[:], sps[:])
            else:
                nc.scalar.copy(ssb[:], sps[:])
            nc.sync.dma_start(scores_out[bc * 128:(bc + 1) * 128, ns:ne],
                              ssb[:])


def build_program(common, per_core, sched, cfg):
    ncores, d, b, n_rel, n_ent = (cfg["ncores"], cfg["d"], cfg["b"],
                                  cfg["n_rel"], cfg["n_ent"])
    T, npc, nblk, npad = sched["T"], sched["npc"], sched["nblk"], sched["npad"]
    nchunk, ocpc, bb = sched["nchunk"], sched["ocpc"], sched["bb"]

    nc = bacc.Bacc("TRN2", target_bir_lowering=False, debug=False,
                   num_devices=ncores)

    di = {}
    def inp(name, arr_shape, dt):
        di[name] = nc.dram_tensor(name, list(arr_shape), dt, kind="ExternalInput")
        return di[name]

    inp("srcT", (128, T), I32); inp("colT", (128, T), F32)
    inp("l1pack", (128, T * 384), BF16)
    inp("l2pack", (128, T * 328), BF16)
    for w in ("W1", "Ws1", "W2", "Ws2"):
        inp(w, (d, d + 1), BF16)
    for w in ("A1m", "A2m", "B1m", "B2m"):
        inp(w, (128, d), F32)
    inp("entT_hi", (128, npad), BF16); inp("entT_lo", (d - 128, npad), BF16)
    inp("bigW", (d, ocpc), BF16)
    inp("rrep_convT", (nchunk * 128, b), BF16)
    inp("acol", (nchunk * 128, 1), F32); inp("ccol", (nchunk * 128, 1), F32)
    inp("acol_a", (nchunk * 128, 1), F32); inp("ccol_a", (nchunk * 128, 1), F32)
    inp("pwc", (ocpc + 1, d), BF16)
    inp("bias_sl", (1, npad), BF16)
    inp("hloc", (128, bb), I32)

    scores_out = nc.dram_tensor("scores", [b, npad], F32, kind="ExternalOutput")

    # internal DRAM (npad slots per core; empty slots carry garbage)
    # ent1 slice is split per AllGather chunk to avoid false WAR deps
    ent1_slk = [nc.dram_tensor(f"ent1_sl{k}", [int(sched["ag_rows"][k]), d],
                               BF16, kind="Internal")
                for k in range(sched["nag"])]
    ent2_sl = nc.dram_tensor("ent2_sl", [npad, d], BF16, kind="Internal")
    z_in = nc.dram_tensor("z_in", [b, d], F32, kind="Internal")
    head_in = nc.dram_tensor("head_in", [b, d], BF16, kind="Internal")
    ent1_full = nc.dram_tensor("ent1_full", [ncores * npad, d], BF16,
                               kind="Internal", addr_space="Shared")
    z_ar = nc.dram_tensor("z_ar", [b, d], F32, kind="Internal",
                          addr_space="Shared")
    head_ar = nc.dram_tensor("head_ar", [b, d], BF16, kind="Internal",
                             addr_space="Shared")

    dch = _dchunks(d)
    with tile.TileContext(nc) as tc:
        with tc.tile_pool(name="cst", bufs=1) as cst, \
             tc.tile_pool(name="epool", bufs=3) as epool, \
             tc.tile_pool(name="vpool", bufs=2) as vpool, \
             tc.tile_pool(name="ps_u", bufs=2, space="PSUM") as ps_u, \
             tc.tile_pool(name="ps_vd", bufs=2, space="PSUM") as ps_vd, \
             tc.tile_pool(name="ps_acc", bufs=2, space="PSUM") as ps_acc:

            ident_bf = cst.tile([128, 128], BF16, tag="ident_bf")
            make_identity(nc, ident_bf[:])
            iota_i = cst.tile([128, 128], I32, tag="iota_i")
            nc.gpsimd.iota(iota_i[:], pattern=[[1, 128]], base=0, channel_multiplier=0)
            iota_f32 = cst.tile([128, 128], F32, tag="iota_f32")
            nc.vector.tensor_copy(iota_f32[:], iota_i[:])
            ones_row = cst.tile([1, bb * 128], BF16, tag="ones_row")
            nc.gpsimd.memset(ones_row[:], 1.0)

            srcT_sb = cst.tile([128, T], I32, tag="idx_srcT")
            nc.sync.dma_start(srcT_sb[:], di["srcT"][:, :])
            colT_sb = cst.tile([128, T], F32, tag="idx_colT")
            nc.sync.dma_start(colT_sb[:], di["colT"][:, :])

            mats = {}
            for nm in ("A1m", "A2m", "B1m", "B2m"):
                mt = cst.tile([128, d], F32, tag=nm)
                nc.sync.dma_start(mt[:], di[nm][:, :])
                mats[nm] = mt
            w_sb = {}
            for nm in ("W1", "W2"):
                w_sb[nm] = []
                for (s, e) in dch:
                    wt = cst.tile([e - s, d + 1], BF16, tag=f"{nm}_{s}")
                    nc.sync.dma_start(wt[:], di[nm][s:e, :])
                    w_sb[nm].append(wt)

            ex_store = cst.tile([128, T], F32, tag="ex_store")
            rd1_store = cst.tile([128, nblk], F32, tag="rd1_store")
            sf1_store = cst.tile([128, nblk], F32, tag="sf1_store")
            v_sb = cst.tile([128, nblk * (d + 1)], BF16, tag="v_sb")
            e1T_hi = cst.tile([128, npad], BF16, tag="e1T_hi")
            e1T_lo = cst.tile([128, npad], BF16, tag="e1T_lo")
            e2T_hi = cst.tile([128, npad], BF16, tag="e2T_hi")
            e2T_lo = cst.tile([128, npad], BF16, tag="e2T_lo")

            # rotating per-tile buffers with constant regions hoisted
            uaug3 = []
            for k in range(3):
                ua = cst.tile([128, 2 * d + 2], BF16, tag=f"uaug{k}")
                nc.gpsimd.memset(ua[:, d + 1:d + 2], 1.0)
                uaug3.append(ua)
            ebf3 = []
            for k in range(3):
                eb = cst.tile([128, 256], BF16, tag=f"ebf{k}")
                nc.gpsimd.memset(eb[:, d:256], 0.0)
                ebf3.append(eb)

            # v1 from host-transposed ent slice
            entT_hi = cst.tile([128, npad], BF16, tag="entT_hi")
            nc.sync.dma_start(entT_hi[:], di["entT_hi"][:, :])
            entT_lo = cst.tile([d - 128, npad], BF16, tag="entT_lo")
            nc.sync.dma_start(entT_lo[:], di["entT_lo"][:, :])
            _emit_v_phase(nc, vpool, ps_u, entT_hi, entT_lo, di["Ws1"], v_sb,
                          nblk, d)

            # ---- layer 1 (AllGather chunks fire as their blocks finish)
            ag = {}
            cob = sched["chunk_of_blk"]
            for k in range(sched["nag"]):
                last_blk = int(np.max(np.nonzero(cob == k)[0]))
                t0 = int(sched["ag_tb"][k])
                t1 = t0 + ncores * int(sched["ag_rows"][k])
                ag[last_blk] = (ent1_full, k, t0, t1)
            _emit_edge_layer(nc, 1, cst, epool, ps_u, ps_vd, ps_acc, sched, di,
                             None, w_sb["W1"], mats["A1m"], mats["B1m"],
                             srcT_sb, colT_sb, iota_f32, v_sb, uaug3, ident_bf,
                             ebf3, ex_store, rd1_store, sf1_store, ent1_slk,
                             e1T_hi, e1T_lo, d, ag=ag)

            # v2 from resident ent1T (overlaps the AllGather)
            _emit_v_phase(nc, vpool, ps_u, e1T_hi, e1T_lo, di["Ws2"], v_sb,
                          nblk, d)

            # ---- layer 2
            _emit_edge_layer(nc, 2, cst, epool, ps_u, ps_vd, ps_acc, sched, di,
                             ent1_full, w_sb["W2"], mats["A2m"], mats["B2m"],
                             srcT_sb, colT_sb, iota_f32, v_sb, uaug3, ident_bf,
                             ebf3, ex_store, rd1_store, sf1_store, ent2_sl,
                             e2T_hi, e2T_lo, d)

            # ---- decoder
            _emit_decoder(nc, cst, vpool, ps_u, sched, di,
                          (ent2_sl, e2T_hi, e2T_lo, head_in, head_ar,
                           z_in, z_ar, scores_out, ident_bf, ones_row),
                          d, b, sched["prelu1"], sched["prelu2"])

    nc.compile()
    return nc


# ---------------------------------------------------------------- entry

_CACHE = {}


def _run(inputs, cfg, sim=False, trace=False):
    common, per_core, sched = _preprocess(inputs, cfg)
    key = (tuple(sorted(cfg.items())), sched["T"], tuple(sched["tpb"]))
    if key not in _CACHE:
        _CACHE[key] = build_program(common, per_core, sched, cfg)
    nc = _CACHE[key]
    in_maps = []
    for c in range(cfg["ncores"]):
        m = dict(common)
        m.update(per_core[c])
        in_maps.append({k: np.ascontiguousarray(v) for k, v in m.items()})
    if sim:
        from concourse.bass_interp import MultiCoreSim
        ms = MultiCoreSim(nc, num_cores=cfg["ncores"])
        for c in range(cfg["ncores"]):
            for name, arr in in_maps[c].items():
                ms.cores[c].tensor(name)[:] = arr
        ms.simulate(check_with_hw=False)
        outs = [np.array(ms.cores[c].tensor("scores")) for c in range(cfg["ncores"])]
        cat = np.concatenate(outs, axis=1)
        return np.ascontiguousarray(cat[:, sched["slot_of"]]), None
    res = bass_utils.run_bass_kernel_spmd(
        nc, in_maps, core_ids=list(range(cfg["ncores"])), trace=trace)
    outs = [res.results[c]["scores"] for c in range(cfg["ncores"])]
    cat = np.concatenate(outs, axis=1).astype(np.float32)
    return np.ascontiguousarray(cat[:, sched["slot_of"]]), res


def kernel(**inputs):
    out, _ = _run(inputs, FULL_CFG)
    return out


# revision 48
# speedup vs baseline: 1.1147x; 1.1147x over previous
"""Trainium2 Bass kernel: 2-layer CompGATv3 encoder + ConvE decoder (KG link scoring).

Sharding (8 NeuronCores, SPMD, full inputs in / full output out):
- Node-parallel GNN: core c owns entity rows [c*6250, (c+1)*6250). Host sorts
  edges by destination and buckets them into 128-node blocks; each block's
  edge list is padded to whole 128-edge tiles (schedule = per-block max over
  cores so one program serves all cores).
- Host prep ships per-edge-slot data so the device never runs indirect
  gathers for layer 1: composed messages (ent_emb[src] * rel_emb[et]) are
  pre-gathered AND pre-transposed (comp1T), the one-hot destination
  selector selT is prebuilt, and rel1/rel2 (tiny 500x200 weight folds) are
  computed on host so layer 2's relation rows (reL2) are pre-gathered too.
  Layer 2 keeps one indirect gather per tile (src rows of the
  device-computed ent1 table).
- The GATv2 destination term v = x @ [Wself | Wself@a] lives in SBUF
  (bf16); the per-edge dest row is produced by a one-hot matmul
  (selT.T @ v_block) instead of a DMA gather.
- Scatter-add into per-block PSUM accumulators via one-hot matmuls; segment
  softmax denominator rides along as a ones-column; epilogue divides,
  blends layer-1 attention (beta), adds bias, tanh, and DMA-transposes the
  block (SBUF->SBUF) into a resident entity-transpose table used by the
  next v-phase and the decoder score matmul.
- Collectives: AllGather of ent1 (layer boundary), small AllReduce of the
  decoder head rows (each core contributes its owned rows; no ent2
  AllGather needed), AllReduce of the projected z.
- Decoder: conv lowered to a sparse matrix; the rrep half of the conv is
  host-folded (rel2[r] is host-known); output-column-sharded over cores
  with a partial-z AllReduce; score matmul uses the resident e2T slice so
  the [B, n_ent] output is column-sharded.
"""

import math
import numpy as np
import ml_dtypes

import concourse.bacc as bacc
import concourse.bass as bass
import concourse.mybir as mybir
import concourse.tile as tile
import concourse.bass_utils as bass_utils
from concourse.bass import IndirectOffsetOnAxis
from concourse.masks import make_identity

F32 = mybir.dt.float32
BF16 = mybir.dt.bfloat16
I32 = mybir.dt.int32
AF = mybir.ActivationFunctionType
OP = mybir.AluOpType
BF16_NP = ml_dtypes.bfloat16

FULL_CFG = dict(n_ent=50000, n_rel=500, d=200, b=256, ncores=8,
                ent_h=10, ent_w=20, fc=32, fs=3)

BETA = 0.5
BN_EPS = 1e-5
LRELU_SLOPE = 0.2
SOFTMAX_EPS = 1e-16
PAD_COL = 999.0
OOB_SENTINEL = 1 << 20


# ---------------------------------------------------------------- host prep

def _ceil_div(a, b):
    return -(-a // b)


def _preprocess(inputs, cfg):
    ncores = cfg["ncores"]
    n_ent, n_rel, d, b = cfg["n_ent"], cfg["n_rel"], cfg["d"], cfg["b"]
    npc = n_ent // ncores
    nblk = _ceil_div(npc, 128)
    npad = nblk * 128

    src = np.asarray(inputs["edge_index"][0], np.int64)
    dst = np.asarray(inputs["edge_index"][1], np.int64)
    et = np.asarray(inputs["edge_type"], np.int64)

    # ---- balanced entity->slot permutation: pack nodes into (core, block)
    # bins of <=128 nodes aiming for <=512 in-edges per bin, so nearly every
    # block needs only ceil(512/128)=4 edge tiles (T ~= 4*nblk instead of the
    # ~5*nblk an unbalanced split needs). Scores are un-permuted on the host.
    import heapq
    deg = np.bincount(dst, minlength=n_ent).astype(np.int64)
    order = np.argsort(-deg, kind="stable")
    core_nodes = [[] for _ in range(ncores)]
    # LPT nodes -> cores (node cap npad per core)
    h1 = [(0, c) for c in range(ncores)]
    heapq.heapify(h1)
    core_load = [0] * ncores
    for n in order:
        while True:
            load, c = heapq.heappop(h1)
            if len(core_nodes[c]) < npad:
                break
        core_nodes[c].append(n)
        core_load[c] = load + int(deg[n])
        if len(core_nodes[c]) < npad:
            heapq.heappush(h1, (core_load[c], c))
    slot_of = np.empty(n_ent, np.int64)
    cap = 4 * 128  # per-bin edge budget for a 4-tile block
    for c in range(ncores):
        nodes = core_nodes[c]
        h2 = [(0, bi) for bi in range(nblk)]
        heapq.heapify(h2)
        bins = [[] for _ in range(nblk)]
        bl = [0] * nblk
        for n in nodes:  # already degree-desc within core
            while True:
                load, bi = heapq.heappop(h2)
                if len(bins[bi]) < 128:
                    break
            bins[bi].append(n)
            bl[bi] = load + int(deg[n])
            if len(bins[bi]) < 128:
                heapq.heappush(h2, (bl[bi], bi))
        # swap-repair: push overloaded bins under cap using underloaded ones
        for _ in range(4 * nblk):
            bo = int(np.argmax(bl))
            if bl[bo] <= cap:
                break
            done = False
            for bu in np.argsort(bl):
                bu = int(bu)
                if bl[bu] >= cap or bu == bo:
                    continue
                need = bl[bo] - cap
                room = cap - bl[bu]
                dbo = np.array([deg[n] for n in bins[bo]])
                dbu = np.array([deg[n] for n in bins[bu]])
                diff = dbo[:, None] - dbu[None, :]
                ok = (diff > 0) & (diff <= room)
                if not ok.any():
                    continue
                gain = np.where(ok, np.minimum(diff, need), -1)
                i, jj = np.unravel_index(np.argmax(gain), gain.shape)
                ni, nj = bins[bo][i], bins[bu][jj]
                bins[bo][i], bins[bu][jj] = nj, ni
                delta = int(deg[ni] - deg[nj])
                bl[bo] -= delta
                bl[bu] += delta
                done = True
                break
            if not done:
                break
        ordb = np.argsort(-np.asarray(bl), kind="stable")
        for newb, oldb in enumerate(ordb):
            for i, n in enumerate(bins[int(oldb)]):
                slot_of[n] = c * npad + newb * 128 + i

    # chunked-AllGather table layout (chunk-major, rank-major within chunk)
    nag = 4
    chunk_of_blk = np.minimum(np.arange(nblk) // _ceil_div(nblk, nag), nag - 1)
    ag_rows = np.array([int((chunk_of_blk == k).sum()) * 128 for k in range(nag)])
    ag_off = np.zeros(nag, np.int64)     # first loc row of chunk
    ag_off[1:] = np.cumsum(ag_rows)[:-1]
    ag_tb = np.zeros(nag, np.int64)      # table base row of chunk
    ag_tb[1:] = np.cumsum(ag_rows * ncores)[:-1]
    sall = np.arange(ncores * npad)
    score_ = sall // npad
    loc_ = sall % npad
    k_ = chunk_of_blk[loc_ // 128]
    trow = ag_tb[k_] + score_ * ag_rows[k_] + (loc_ - ag_off[k_])

    dslot = slot_of[dst]
    core_of = dslot // npad
    cnts = np.zeros((ncores, nblk), np.int64)
    percore = []
    for c in range(ncores):
        m = core_of == c
        s_c, t_c = src[m], et[m]
        loc = (dslot[m] - c * npad).astype(np.int64)
        o = np.argsort(loc, kind="stable")
        s_c, t_c, loc = s_c[o], t_c[o], loc[o]
        blk = loc // 128
        cnts[c] = np.bincount(blk, minlength=nblk)
        percore.append((s_c, t_c, loc, blk))

    tpb = np.maximum(1, _ceil_div(cnts.max(axis=0), 128)).astype(np.int64)
    T = int(tpb.sum())
    tile_blk = np.repeat(np.arange(nblk), tpb)
    tstart = np.zeros(nblk, np.int64)
    tstart[1:] = np.cumsum(tpb)[:-1]

    f32 = lambda x: np.ascontiguousarray(np.asarray(x, np.float32))
    bf = lambda x: np.ascontiguousarray(np.asarray(x, np.float32).astype(BF16_NP))

    ent_emb = f32(inputs["ent_emb"])
    rel_emb = f32(inputs["rel_emb"])
    rel1 = rel_emb @ f32(inputs["Wrel1"])
    rel2 = rel1 @ f32(inputs["Wrel2"])

    # one packed DMA per edge tile per layer:
    # l1pack tile cols [0:128]=comp1T_hi, [128:256] rows0:72=comp1T_lo,
    #   [256:384]=selT;  l2pack tile cols [0:200]=re rows, [200:328]=selT
    srcT = np.zeros((ncores, 128, T), np.int32)
    colT = np.full((ncores, 128, T), PAD_COL, np.float32)
    l1pack = np.zeros((ncores, 128, T * 384), BF16_NP)
    l2pack = np.zeros((ncores, 128, T * 328), BF16_NP)
    for c in range(ncores):
        s_c, t_c, loc, blk = percore[c]
        off = np.zeros(nblk, np.int64)
        off[1:] = np.cumsum(cnts[c])[:-1]
        wb = np.arange(len(s_c)) - off[blk]          # index within block
        slot = tstart[blk] * 128 + wb                # flat slot in [T*128]
        fs_ = np.zeros(T * 128, np.int32)
        fc_ = np.full(T * 128, PAD_COL, np.float32)
        fs_[slot] = trow[slot_of[s_c]]
        fc_[slot] = (loc % 128).astype(np.float32)
        srcT[c] = fs_.reshape(T, 128).T
        colT[c] = fc_.reshape(T, 128).T
        comp1 = np.zeros((T * 128, d), np.float32)
        comp1[slot] = ent_emb[s_c] * rel_emb[t_c]
        c1t = comp1.T.astype(BF16_NP)                # [d, T*128]
        re2 = np.zeros((T * 128, d), np.float32)
        re2[slot] = rel1[t_c]
        re2 = re2.astype(BF16_NP)
        st = np.zeros((128, T * 128), np.float32)
        st[(loc % 128).astype(np.int64), slot] = 1.0
        st = st.astype(BF16_NP)
        p1 = l1pack[c].reshape(128, T, 384)
        p1[:, :, 0:128] = c1t[:128].reshape(128, T, 128)
        p1[:d - 128, :, 128:256] = c1t[128:].reshape(d - 128, T, 128)
        p1[:, :, 256:384] = st.reshape(128, T, 128)
        p2 = l2pack[c].reshape(128, T, 328)
        p2[:, :, 0:d] = re2.reshape(T, 128, d).transpose(1, 0, 2)
        p2[:, :, d:328] = st.reshape(128, T, 128)

    def aug(w, a):
        # [d, d+1]: last column is w @ a (linear part of the attention logit)
        w = f32(w)
        return np.concatenate([w, (w @ f32(a))[:, None]], axis=1)

    entT = []
    for c in range(ncores):
        m = (slot_of // npad) == c
        locs = slot_of[m] - c * npad
        sl = np.zeros((d, npad), np.float32)
        sl[:, locs] = ent_emb[m].T
        entT.append(bf(sl))

    # ---- decoder prep
    ent_h, ent_w, fc, fs_k = cfg["ent_h"], cfg["ent_w"], cfg["fc"], cfg["fs"]
    hh, ww = 2 * ent_h, ent_w                 # image dims (20, 20)
    oh, ow = hh - fs_k + 1, ww - fs_k + 1     # conv output (18, 18)
    num_in = fc * oh * ow
    npix = hh * ww                            # 400
    conv_w = f32(inputs["conv_w"])            # [fc, 1, fs, fs]
    g0p = float(np.asarray(inputs["bn0_g"], np.float32)[0] / math.sqrt(1.0 + BN_EPS))
    b0 = float(np.asarray(inputs["bn0_b"], np.float32)[0])
    g1p = f32(inputs["bn1_g"]) / math.sqrt(1.0 + BN_EPS)
    b1v = f32(inputs["bn1_b"])
    gpp = f32(inputs["bnp_g"]) / math.sqrt(1.0 + BN_EPS)
    bpv = f32(inputs["bnp_b"])
    prelu1 = float(np.asarray(inputs["prelu1"], np.float32).ravel()[0])
    prelu2 = float(np.asarray(inputs["prelu2"], np.float32).ravel()[0])

    big_w = np.zeros((npix, num_in), np.float32)
    oy, ox = np.meshgrid(np.arange(oh), np.arange(ow), indexing="ij")
    for oc in range(fc):
        for dy in range(fs_k):
            for dx in range(fs_k):
                pix = (oy + dy) * ww + (ox + dx)
                out_i = oc * (oh * ow) + oy * ow + ox
                big_w[pix, out_i] = conv_w[oc, 0, dy, dx] * g0p
    # pixel reorder: [head dims 0..d-1, tail dims 0..d-1] (orig interleaved 2d, 2d+1)
    perm = np.concatenate([np.arange(d) * 2, np.arange(d) * 2 + 1])
    big_w = big_w[perm]

    ridx = np.asarray(inputs["r"], np.int64)
    rrep = rel2[ridx]                         # [B, d] host-known

    ocpc = num_in // ncores          # out-columns per core
    occ = fc // ncores               # conv channels per core
    sumw = conv_w.reshape(fc, -1).sum(1)
    nchunk = _ceil_div(ocpc, 128)
    acol = np.zeros((ncores, nchunk * 128, 1), np.float32)
    ccol = np.zeros((ncores, nchunk * 128, 1), np.float32)
    rrep_convT = np.zeros((ncores, nchunk * 128, b), BF16_NP)
    for c in range(ncores):
        ocs = np.arange(ocpc) // (oh * ow) + c * occ
        acol[c, :ocpc, 0] = g1p[ocs]
        ccol[c, :ocpc, 0] = g1p[ocs] * b0 * sumw[ocs] + b1v[ocs]
        rc = rrep @ big_w[d:, c * ocpc:(c + 1) * ocpc]   # [B, ocpc]
        rrep_convT[c, :ocpc] = rc.T.astype(BF16_NP)

    acol_a = acol * prelu1           # scale/bias for the linear branch of prelu
    ccol_a = ccol * prelu1

    pw = f32(inputs["proj_w"]) * gpp[None, :]
    pb = f32(inputs["proj_b"]) * gpp + bpv
    pwc = np.zeros((ncores, ocpc + 1, d), np.float32)
    for c in range(ncores):
        pwc[c, :ocpc] = pw[c * ocpc:(c + 1) * ocpc]
    pwc[0, ocpc] = pb                      # bias row only on core 0 (AllReduce sums)

    bias_ent = f32(inputs["bias_ent"])
    bias_sl = np.zeros((ncores, 1, npad), np.float32)
    for c in range(ncores):
        m = (slot_of // npad) == c
        bias_sl[c, 0, slot_of[m] - c * npad] = bias_ent[m]

    hidx = np.asarray(inputs["h"], np.int64)
    hslot = slot_of[hidx]
    bb = b // 128                           # batch chunks (2)
    hloc = np.full((ncores, 128, bb), OOB_SENTINEL, np.int32)
    for c in range(ncores):
        own = (hslot // npad) == c
        hl = np.where(own, hslot - c * npad, OOB_SENTINEL).astype(np.int32)
        hloc[c] = hl.reshape(bb, 128).T

    common = {
        "W1": bf(aug(inputs["W1"], inputs["a1"])),
        "Ws1": bf(aug(inputs["Wself1"], inputs["a1"])),
        "W2": bf(aug(inputs["W2"], inputs["a2"])),
        "Ws2": bf(aug(inputs["Wself2"], inputs["a2"])),
        "A1m": f32(np.broadcast_to(np.asarray(inputs["a1"], np.float32), (128, d))),
        "A2m": f32(np.broadcast_to(np.asarray(inputs["a2"], np.float32), (128, d))),
        "B1m": f32(np.broadcast_to(np.asarray(inputs["b1"], np.float32), (128, d))),
        "B2m": f32(np.broadcast_to(np.asarray(inputs["b2"], np.float32), (128, d))),
    }
    per_core = []
    for c in range(ncores):
        per_core.append({
            "srcT": srcT[c], "colT": colT[c],
            "l1pack": l1pack[c], "l2pack": l2pack[c],
            "entT_hi": entT[c][:128], "entT_lo": entT[c][128:d],
            "bigW": bf(big_w[:d, c * ocpc:(c + 1) * ocpc]),
            "rrep_convT": rrep_convT[c],
            "acol": acol[c], "ccol": ccol[c],
            "acol_a": acol_a[c], "ccol_a": ccol_a[c],
            "pwc": bf(pwc[c]),
            "bias_sl": bf(bias_sl[c]),
            "hloc": hloc[c],
        })

    sched = dict(T=T, tpb=tpb, tile_blk=tile_blk, tstart=tstart,
                 npc=npad, nblk=nblk, npad=npad, nchunk=nchunk, ocpc=ocpc,
                 prelu1=prelu1, prelu2=prelu2, npix=npix, bb=bb,
                 slot_of=slot_of, nag=nag, chunk_of_blk=chunk_of_blk,
                 ag_rows=ag_rows, ag_off=ag_off, ag_tb=ag_tb)
    return common, per_core, sched


# ---------------------------------------------------------------- device code

def _dchunks(d):
    out = []
    s = 0
    while s < d:
        out.append((s, min(s + 128, d)))
        s += 128
    return out


def _emit_v_phase(nc, pool, ps_pool, xT_hi, xT_lo, w_dram, v_sb, nblk, d):
    """v_sb[:, blk] = x[n,:] @ [Wself | Wself@a] for owned nodes (bf16 SBUF)."""
    dch = _dchunks(d)
    w_sb = []
    for (s, e) in dch:
        wt = pool.tile([e - s, d + 1], BF16, tag=f"vw{s}")
        nc.sync.dma_start(wt[:], w_dram[s:e, :])
        w_sb.append(wt)
    for bkt in range(nblk):
        vps = ps_pool.tile([128, d + 1], F32, tag="ups")
        nc.tensor.matmul(vps[:], lhsT=xT_hi[0:128, bkt * 128:(bkt + 1) * 128],
                         rhs=w_sb[0][:], start=True, stop=False)
        nc.tensor.matmul(vps[:], lhsT=xT_lo[0:d - 128, bkt * 128:(bkt + 1) * 128],
                         rhs=w_sb[1][:], start=False, stop=True)
        nc.vector.tensor_copy(v_sb[:, bkt * (d + 1):(bkt + 1) * (d + 1)], vps[:])


def _emit_edge_layer(nc, layer, cst, epool, ps_u, ps_vd, ps_acc, sched, di,
                     x_tab, w_sb, amat, bmat, srcT_sb, colT_sb, iota_f32,
                     v_sb, uaug3, ident_bf, ebf3, ex_store, rd1_store,
                     sf1_store, ent_sl, eT_hi, eT_lo, d, ag=None):
    """One CompGAT layer over all edge tiles + per-block epilogues.

    The destination self-term rides in the message PSUM (third K-chunk via
    the one-hot selT), so the accumulated scatter output is sum_e w_e*z_e;
    the epilogue subtracts v_n * (normalized weight sum) to recover
    sum_e w_e*msg_e exactly (also correct for isolated nodes).
    """
    T, tpb, tile_blk, tstart = sched["T"], sched["tpb"], sched["tile_blk"], sched["tstart"]
    dlo = d - 128
    pend = []
    delay = 1 if layer == 1 else 0
    state = {}

    def _flush(item):
        _emit_scatter_epilogue(nc, layer, epool, ps_acc, sched, state, bmat,
                               colT_sb, v_sb, ebf3, rd1_store, sf1_store,
                               ent_sl, eT_hi, eT_lo, d, ag, item)

    for t in range(T):
        bkt = int(tile_blk[t])
        j = t - int(tstart[bkt])
        last = j == int(tpb[bkt]) - 1

        if layer == 1:
            pk = epool.tile([128, 384], BF16, tag="pk")
            nc.sync.dma_start(pk[:], di["l1pack"][:, t * 384:(t + 1) * 384])
            lhsA, lhsB = pk[:, 0:128], pk[0:dlo, 128:256]
            sT = pk[:, 256:384]
        else:
            pk = epool.tile([128, 328], BF16, tag="pk")
            nc.sync.dma_start(pk[:], di["l2pack"][:, t * 328:(t + 1) * 328])
            sT = pk[:, d:328]
            xs = epool.tile([128, d], BF16, tag="xs")
            nc.gpsimd.indirect_dma_start(
                out=xs[:], out_offset=None, in_=x_tab[:, :],
                in_offset=IndirectOffsetOnAxis(ap=srcT_sb[:, t:t + 1], axis=0))
            cm = epool.tile([128, d], BF16, tag="cm")
            nc.vector.tensor_tensor(out=cm[:], in0=xs[:], in1=pk[:, 0:d], op=OP.mult)
            trA = ps_vd.tile([128, 128], BF16, tag="vd")
            nc.tensor.transpose(out=trA[:], in_=cm[:, 0:128], identity=ident_bf[:])
            ctA = epool.tile([128, 128], BF16, tag="ctA")
            nc.vector.tensor_copy(ctA[:], trA[:])
            trB = ps_vd.tile([128, 128], BF16, tag="vd")
            nc.tensor.transpose(out=trB[0:dlo, :], in_=cm[:, 128:d], identity=ident_bf[:])
            ctB = epool.tile([dlo, 128], BF16, tag="ctB")
            nc.scalar.copy(ctB[:], trB[0:dlo, :])
            lhsA, lhsB = ctA[:, :], ctB[:, :]

        # z = (x_src*rel) @ Waug + v_dst, all in one PSUM accumulation
        ups = ps_u.tile([128, d + 1], F32, tag="ups")
        nc.tensor.matmul(ups[:], lhsT=lhsA, rhs=w_sb[0][:], start=True, stop=False)
        nc.tensor.matmul(ups[:], lhsT=lhsB, rhs=w_sb[1][:], start=False, stop=False)
        nc.tensor.matmul(ups[:], lhsT=sT,
                         rhs=v_sb[:, bkt * (d + 1):(bkt + 1) * (d + 1)],
                         start=False, stop=True)
        ua = uaug3[t % 3]
        nc.scalar.copy(ua[:, 0:d + 1], ups[:])
        # lrelu(z)@a = c1*(z@a) + c2*(|z|@a), c1=(1+s)/2, c2=(1-s)/2
        abz = epool.tile([128, d], F32, tag="abz")
        nc.scalar.activation(abz[:], ups[:, 0:d], AF.Abs)
        e0 = epool.tile([128, 1], F32, tag="e0")
        nc.vector.tensor_scalar(out=e0[:], in0=ups[:, d:d + 1],
                                scalar1=(1.0 + LRELU_SLOPE) / 2.0,
                                scalar2=None, op0=OP.mult)
        ttro = epool.tile([128, d], F32, tag="ttro")
        e_sb = epool.tile([128, 1], F32, tag="esb")
        nc.vector.scalar_tensor_tensor(out=ttro[:], in0=abz[:], scalar=1.0,
                                       in1=amat[:], op0=OP.mult, op1=OP.mult,
                                       accum_out=e_sb[:])
        if layer == 1:
            ex_ap = ex_store[:, t:t + 1]
        else:
            ex_t = epool.tile([128, 1], F32, tag="ex2")
            ex_ap = ex_t[:, :]
        nc.scalar.activation(ex_ap, e_sb[:], AF.Exp,
                             scale=(1.0 - LRELU_SLOPE) / 2.0, bias=e0[:, 0:1])
        if layer == 1:
            # one-hot scatter, lhsT carries the ex1 weight
            sex = epool.tile([128, 128], BF16, tag="sex")
            nc.vector.tensor_scalar(out=sex[:], in0=iota_f32[:],
                                    scalar1=colT_sb[:, t:t + 1], scalar2=ex_ap,
                                    op0=OP.is_equal, op1=OP.mult)
            rhs_ap = ua[:, 0:d + 2]
        else:
            # single scatter with stacked rhs: [ex2*(z|junk|1) | ex1*z]
            sex = epool.tile([128, 128], BF16, tag="sex")
            nc.vector.tensor_scalar(out=sex[:], in0=iota_f32[:],
                                    scalar1=colT_sb[:, t:t + 1], scalar2=None,
                                    op0=OP.is_equal)
            ua2 = epool.tile([128, 2 * d + 2], BF16, tag="ua2")
            nc.vector.tensor_scalar(out=ua2[:, 0:d + 2], in0=ua[:, 0:d + 2],
                                    scalar1=ex_ap, scalar2=None, op0=OP.mult)
            nc.scalar.activation(ua2[:, d + 2:2 * d + 2], ua[:, 0:d], AF.Identity,
                                 scale=ex_store[:, t:t + 1])
            rhs_ap = ua2[:, :]
        # scatter + epilogue are deferred one tile so the tensor engine can
        # start the next tile's matmuls while this tile's logit chain runs
        pend.append((sex, rhs_ap, bkt, j, last))
        if len(pend) == delay + 1:
            _flush(pend.pop(0))
    while pend:
        _flush(pend.pop(0))


def _emit_scatter_epilogue(nc, layer, epool, ps_acc, sched, state, bmat,
                           colT_sb, v_sb, ebf3, rd1_store, sf1_store,
                           ent_sl, eT_hi, eT_lo, d, ag, item):
    sex, rhs_ap, bkt, j, last = item
    if j == 0:
        w = (d + 2) if layer == 1 else (2 * d + 2)
        state["acc"] = ps_acc.tile([128, w], F32, tag="acc", name="acc")
    acc = state["acc"]
    nc.tensor.matmul(acc[:], lhsT=sex[:], rhs=rhs_ap,
                     start=(j == 0), stop=last)
    if last:
            vblk = v_sb[:, bkt * (d + 1):bkt * (d + 1) + d]
            den_eps = epool.tile([128, 1], F32, tag="deneps")
            nc.vector.tensor_scalar(out=den_eps[:], in0=acc[:, d + 1:d + 2],
                                    scalar1=SOFTMAX_EPS, scalar2=None, op0=OP.add)
            if layer == 1:
                rd_ap = rd1_store[:, bkt:bkt + 1]
                nc.vector.reciprocal(rd_ap, den_eps[:])
                # sf1 = den1*rd1 (1 if node has edges else 0)
                nc.vector.tensor_scalar(out=sf1_store[:, bkt:bkt + 1],
                                        in0=acc[:, d + 1:d + 2], scalar1=rd_ap,
                                        scalar2=None, op0=OP.mult)
                t1 = epool.tile([128, d], F32, tag="ep_t1")
                nc.vector.tensor_scalar(out=t1[:], in0=acc[:, 0:d], scalar1=rd_ap,
                                        scalar2=None, op0=OP.mult)
                tcor = epool.tile([128, d], F32, tag="ep_tc")
                nc.vector.tensor_scalar(out=tcor[:], in0=vblk,
                                        scalar1=sf1_store[:, bkt:bkt + 1],
                                        scalar2=None, op0=OP.mult)
            else:
                rd2 = epool.tile([128, 1], F32, tag="rd2")
                nc.vector.reciprocal(rd2[:], den_eps[:])
                sf2 = epool.tile([128, 1], F32, tag="sf2")
                nc.vector.tensor_scalar(out=sf2[:], in0=acc[:, d + 1:d + 2],
                                        scalar1=rd2[:, :], scalar2=1.0 - BETA,
                                        op0=OP.mult, op1=OP.mult)
                cf = epool.tile([128, 1], F32, tag="cf")
                nc.vector.tensor_scalar(out=cf[:], in0=sf1_store[:, bkt:bkt + 1],
                                        scalar1=BETA, scalar2=sf2[:, :],
                                        op0=OP.mult, op1=OP.add)
                tB = epool.tile([128, d], F32, tag="ep_tB")
                nc.vector.tensor_scalar(out=tB[:], in0=acc[:, 0:d], scalar1=rd2[:, :],
                                        scalar2=1.0 - BETA, op0=OP.mult, op1=OP.mult)
                tA = epool.tile([128, d], F32, tag="ep_tA")
                nc.vector.tensor_scalar(out=tA[:], in0=acc[:, d + 2:2 * d + 2],
                                        scalar1=rd1_store[:, bkt:bkt + 1],
                                        scalar2=BETA, op0=OP.mult, op1=OP.mult)
                t1 = epool.tile([128, d], F32, tag="ep_t1")
                nc.vector.tensor_tensor(out=t1[:], in0=tA[:], in1=tB[:], op=OP.add)
                tcor = epool.tile([128, d], F32, tag="ep_tc")
                nc.vector.tensor_scalar(out=tcor[:], in0=vblk,
                                        scalar1=cf[:, :], scalar2=None, op0=OP.mult)
            tsub = epool.tile([128, d], F32, tag="ep_ts")
            nc.vector.tensor_tensor(out=tsub[:], in0=t1[:], in1=tcor[:], op=OP.subtract)
            t2 = epool.tile([128, d], F32, tag="ep_t2")
            nc.vector.tensor_tensor(out=t2[:], in0=tsub[:], in1=bmat[:], op=OP.add)
            ebf = ebf3[bkt % 3]
            nc.scalar.activation(ebf[:, 0:d], t2[:], AF.Tanh)
            if isinstance(ent_sl, list):
                k = int(sched["chunk_of_blk"][bkt])
                row = bkt * 128 - int(sched["ag_off"][k])
                nc.sync.dma_start(ent_sl[k][row:row + 128, :], ebf[:, 0:d])
            else:
                nc.sync.dma_start(ent_sl[bkt * 128:(bkt + 1) * 128, :],
                                  ebf[:, 0:d])
            nc.sync.dma_start_transpose(
                eT_hi[:, bkt * 128:(bkt + 1) * 128], ebf[:, 0:128])
            nc.scalar.dma_start_transpose(
                eT_lo[:, bkt * 128:(bkt + 1) * 128], ebf[:, 128:256])
            if ag is not None and bkt in ag:
                ful, kk, t0, t1 = ag[bkt]
                nc.gpsimd.collective_compute(
                    "AllGather", OP.bypass,
                    replica_groups=[list(range(FULL_CFG["ncores"]))],
                    ins=[ent_sl[kk][:, :]], outs=[ful[t0:t1, :]])


def _emit_decoder(nc, cst, pool, ps_pool, sched, di, tensors, d, b, prelu1, prelu2):
    npad, nchunk, ocpc, bb = sched["npad"], sched["nchunk"], sched["ocpc"], sched["bb"]
    npc = sched["npc"]
    (ent2_sl, e2T_hi, e2T_lo, head_in, head_ar, z_in, z_ar, scores_out,
     ident_bf, ones_row) = tensors
    dch = _dchunks(d)
    dlo = d - 128

    hloc_sb = cst.tile([128, bb], I32, tag="hloc")
    nc.sync.dma_start(hloc_sb[:], di["hloc"][:, :])

    # each core contributes its owned head rows; AllReduce assembles [B, d]
    for bc in range(bb):
        hp = pool.tile([128, d], BF16, tag="dec_hp")
        nc.gpsimd.memset(hp[:], 0.0)
        nc.gpsimd.indirect_dma_start(
            out=hp[:], out_offset=None, in_=ent2_sl[:, :],
            in_offset=IndirectOffsetOnAxis(ap=hloc_sb[:, bc:bc + 1], axis=0),
            bounds_check=npad - 1, oob_is_err=False)
        nc.sync.dma_start(head_in[bc * 128:(bc + 1) * 128, :], hp[:])
    nc.gpsimd.collective_compute(
        "AllReduce", OP.add, replica_groups=[list(range(FULL_CFG["ncores"]))],
        ins=[head_in.ap()], outs=[head_ar.ap()])

    # head imgT K-chunks [d-chunks x b] via PE transposes
    imgT = []
    for (s, e) in dch:
        t = cst.tile([e - s, bb * 128], BF16, tag=f"imgTh{s}")
        imgT.append(t)
    for bc in range(bb):
        head = pool.tile([128, d], BF16, tag="dec_head")
        nc.sync.dma_start(head[:], head_ar[bc * 128:(bc + 1) * 128, :])
        for i, (s, e) in enumerate(dch):
            tp = ps_pool.tile([128, 128], BF16, tag="ups")
            nc.tensor.transpose(out=tp[0:e - s, 0:128], in_=head[:, s:e],
                                identity=ident_bf[:])
            nc.scalar.copy(imgT[i][:, bc * 128:(bc + 1) * 128], tp[0:e - s, 0:128])

    # conv via sparse matrix: head K-chunks on device, rrep half host-folded
    bw_sb = []
    for i, (s, e) in enumerate(dch):
        t = cst.tile([e - s, ocpc], BF16, tag=f"bw{i}")
        nc.sync.dma_start(t[:], di["bigW"][s:e, :])
        bw_sb.append(t)
    acol_sb = cst.tile([128, nchunk], F32, tag="acol")
    nc.sync.dma_start(acol_sb[:], di["acol"][:, :].rearrange("(c p) o -> p (c o)", p=128))
    ccol_sb = cst.tile([128, nchunk], F32, tag="ccol")
    nc.sync.dma_start(ccol_sb[:], di["ccol"][:, :].rearrange("(c p) o -> p (c o)", p=128))
    acola_sb = cst.tile([128, nchunk], F32, tag="acola")
    nc.sync.dma_start(acola_sb[:], di["acol_a"][:, :].rearrange("(c p) o -> p (c o)", p=128))
    ccola_sb = cst.tile([128, nchunk], F32, tag="ccola")
    nc.sync.dma_start(ccola_sb[:], di["ccol_a"][:, :].rearrange("(c p) o -> p (c o)", p=128))

    yT = []
    for ci in range(nchunk):
        cols = min(128, ocpc - ci * 128)
        yt = cst.tile([cols, bb * 128], BF16, tag=f"yT{ci}")
        cps = ps_pool.tile([128, bb * 128], F32, tag="ups")
        nc.tensor.matmul(cps[0:cols, :], lhsT=bw_sb[0][:, ci * 128:ci * 128 + cols],
                         rhs=imgT[0][:], start=True, stop=False)
        nc.tensor.matmul(cps[0:cols, :], lhsT=bw_sb[1][:, ci * 128:ci * 128 + cols],
                         rhs=imgT[1][:], start=False, stop=True)
        rT = pool.tile([128, bb * 128], BF16, tag="dec_rT")
        nc.sync.dma_start(rT[:], di["rrep_convT"][ci * 128:(ci + 1) * 128, :])
        tmp = pool.tile([128, bb * 128], F32, tag="dec_tmp")
        nc.vector.tensor_tensor(out=tmp[0:cols, :], in0=cps[0:cols, :],
                                in1=rT[0:cols, :], op=OP.add)
        # prelu(w) = a*w + (1-a)*relu(w), w = A*conv + C
        wlin = pool.tile([128, bb * 128], F32, tag="dec_wlin")
        nc.scalar.activation(wlin[0:cols, :], tmp[0:cols, :], AF.Identity,
                             scale=acola_sb[0:cols, ci:ci + 1],
                             bias=ccola_sb[0:cols, ci:ci + 1])
        wrel = pool.tile([128, bb * 128], F32, tag="dec_wrel")
        nc.scalar.activation(wrel[0:cols, :], tmp[0:cols, :], AF.Relu,
                             scale=acol_sb[0:cols, ci:ci + 1],
                             bias=ccol_sb[0:cols, ci:ci + 1])
        wrs = pool.tile([128, bb * 128], F32, tag="dec_wrs")
        nc.vector.tensor_scalar(out=wrs[0:cols, :], in0=wrel[0:cols, :],
                                scalar1=1.0 - prelu1, scalar2=None, op0=OP.mult)
        nc.vector.tensor_tensor(out=yt[0:cols, :], in0=wlin[0:cols, :],
                                in1=wrs[0:cols, :], op=OP.add)
        yT.append(yt)

    # proj: z[b, d] partial = sum_ci yT_ci.T @ pw_ci  + ones.T @ pb (core 0 only)
    pbrow = cst.tile([1, d], BF16, tag="pbrow")
    nc.sync.dma_start(pbrow[:], di["pwc"][ocpc:ocpc + 1, :])
    pw_sb = []
    for ci in range(nchunk):
        cols = min(128, ocpc - ci * 128)
        pwt = cst.tile([cols, d], BF16, tag=f"pw{ci}", name=f"pw{ci}")
        nc.sync.dma_start(pwt[:], di["pwc"][ci * 128:ci * 128 + cols, :])
        pw_sb.append(pwt)
    for bc in range(bb):
        zps = ps_pool.tile([128, d], F32, tag="ups")
        for ci in range(nchunk):
            nc.tensor.matmul(zps[:], lhsT=yT[ci][:, bc * 128:(bc + 1) * 128],
                             rhs=pw_sb[ci][:], start=(ci == 0), stop=False)
        nc.tensor.matmul(zps[:], lhsT=ones_row[0:1, bc * 128:(bc + 1) * 128],
                         rhs=pbrow[:], start=False, stop=True)
        zsb = pool.tile([128, d], F32, tag="dec_zsb")
        nc.vector.tensor_copy(zsb[:], zps[:])
        nc.sync.dma_start(z_in[bc * 128:(bc + 1) * 128, :], zsb[:])

    nc.gpsimd.collective_compute(
        "AllReduce", OP.add, replica_groups=[list(range(FULL_CFG["ncores"]))],
        ins=[z_in.ap()], outs=[z_ar.ap()])

    # prelu2 + transpose z2
    z2 = pool.tile([128, bb * d], F32, tag="z2")
    for bc in range(bb):
        nc.sync.dma_start(z2[:, bc * d:(bc + 1) * d], z_ar[bc * 128:(bc + 1) * 128, :])
    z2r = pool.tile([128, bb * d], F32, tag="z2r")
    nc.scalar.activation(z2r[:], z2[:], AF.Relu, scale=1.0 - prelu2)
    z2l = pool.tile([128, bb * d], F32, tag="z2l")
    nc.vector.tensor_scalar(out=z2l[:], in0=z2[:], scalar1=prelu2, scalar2=None,
                            op0=OP.mult)
    z2p = pool.tile([128, bb * d], BF16, tag="z2p")
    nc.vector.tensor_tensor(out=z2p[:], in0=z2l[:], in1=z2r[:], op=OP.add)
    z2T_hi = cst.tile([128, bb * 128], BF16, tag="z2T_hi")
    z2T_lo = cst.tile([dlo, bb * 128], BF16, tag="z2T_lo")
    for bc in range(bb):
        for i, (s, e) in enumerate(dch):
            tp = ps_pool.tile([128, 128], BF16, tag="ups")
            nc.tensor.transpose(out=tp[0:e - s, 0:128],
                                in_=z2p[:, bc * d + s:bc * d + e], identity=ident_bf[:])
            tgt = z2T_hi if i == 0 else z2T_lo
            nc.scalar.copy(tgt[0:e - s, bc * 128:(bc + 1) * 128], tp[0:e - s, 0:128])

    biasrow = cst.tile([1, npad], BF16, tag="biasrow")
    nc.sync.dma_start(biasrow[:], di["bias_sl"][0:1, :])

    for ns in range(0, npad, 512):
        ne = min(ns + 512, npad)
        for bc in range(bb):
            sps = ps_pool.tile([128, ne - ns], F32, tag="ups")
            nc.tensor.matmul(sps[:], lhsT=z2T_hi[:, bc * 128:(bc + 1) * 128],
                             rhs=e2T_hi[:, ns:ne], start=True, stop=False)
            nc.tensor.matmul(sps[:], lhsT=z2T_lo[:, bc * 128:(bc + 1) * 128],
                             rhs=e2T_lo[0:dlo, ns:ne], start=False, stop=False)
            nc.tensor.matmul(sps[:], lhsT=ones_row[0:1, bc * 128:(bc + 1) * 128],
                             rhs=biasrow[0:1, ns:ne], start=False, stop=True)
            ssb = pool.tile([128, ne - ns], F32, tag="dec_ssb")
            if bc % 2 == 0:
                nc.vector.tensor_copy(ssb[:], sps[:])
            else:
                nc.scalar.copy(ssb[:], sps[:])
            nc.sync.dma_start(scores_out[bc * 128:(bc + 1) * 128, ns:ne],
                              ssb[:])


def build_program(common, per_core, sched, cfg):
    ncores, d, b, n_rel, n_ent = (cfg["ncores"], cfg["d"], cfg["b"],
                                  cfg["n_rel"], cfg["n_ent"])
    T, npc, nblk, npad = sched["T"], sched["npc"], sched["nblk"], sched["npad"]
    nchunk, ocpc, bb = sched["nchunk"], sched["ocpc"], sched["bb"]

    nc = bacc.Bacc("TRN2", target_bir_lowering=False, debug=False,
                   num_devices=ncores)

    di = {}
    def inp(name, arr_shape, dt):
        di[name] = nc.dram_tensor(name, list(arr_shape), dt, kind="ExternalInput")
        return di[name]

    inp("srcT", (128, T), I32); inp("colT", (128, T), F32)
    inp("l1pack", (128, T * 384), BF16)
    inp("l2pack", (128, T * 328), BF16)
    for w in ("W1", "Ws1", "W2", "Ws2"):
        inp(w, (d, d + 1), BF16)
    for w in ("A1m", "A2m", "B1m", "B2m"):
        inp(w, (128, d), F32)
    inp("entT_hi", (128, npad), BF16); inp("entT_lo", (d - 128, npad), BF16)
    inp("bigW", (d, ocpc), BF16)
    inp("rrep_convT", (nchunk * 128, b), BF16)
    inp("acol", (nchunk * 128, 1), F32); inp("ccol", (nchunk * 128, 1), F32)
    inp("acol_a", (nchunk * 128, 1), F32); inp("ccol_a", (nchunk * 128, 1), F32)
    inp("pwc", (ocpc + 1, d), BF16)
    inp("bias_sl", (1, npad), BF16)
    inp("hloc", (128, bb), I32)

    scores_out = nc.dram_tensor("scores", [b, npad], F32, kind="ExternalOutput")

    # internal DRAM (npad slots per core; empty slots carry garbage)
    # ent1 slice is split per AllGather chunk to avoid false WAR deps
    ent1_slk = [nc.dram_tensor(f"ent1_sl{k}", [int(sched["ag_rows"][k]), d],
                               BF16, kind="Internal")
                for k in range(sched["nag"])]
    ent2_sl = nc.dram_tensor("ent2_sl", [npad, d], BF16, kind="Internal")
    z_in = nc.dram_tensor("z_in", [b, d], F32, kind="Internal")
    head_in = nc.dram_tensor("head_in", [b, d], BF16, kind="Internal")
    ent1_full = nc.dram_tensor("ent1_full", [ncores * npad, d], BF16,
                               kind="Internal", addr_space="Shared")
    z_ar = nc.dram_tensor("z_ar", [b, d], F32, kind="Internal",
                          addr_space="Shared")
    head_ar = nc.dram_tensor("head_ar", [b, d], BF16, kind="Internal",
                             addr_space="Shared")

    dch = _dchunks(d)
    with tile.TileContext(nc) as tc:
        with tc.tile_pool(name="cst", bufs=1) as cst, \
             tc.tile_pool(name="epool", bufs=3) as epool, \
             tc.tile_pool(name="vpool", bufs=2) as vpool, \
             tc.tile_pool(name="ps_u", bufs=2, space="PSUM") as ps_u, \
             tc.tile_pool(name="ps_vd", bufs=2, space="PSUM") as ps_vd, \
             tc.tile_pool(name="ps_acc", bufs=2, space="PSUM") as ps_acc:

            ident_bf = cst.tile([128, 128], BF16, tag="ident_bf")
            make_identity(nc, ident_bf[:])
            iota_i = cst.tile([128, 128], I32, tag="iota_i")
            nc.gpsimd.iota(iota_i[:], pattern=[[1, 128]], base=0, channel_multiplier=0)
            iota_f32 = cst.tile([128, 128], F32, tag="iota_f32")
            nc.vector.tensor_copy(iota_f32[:], iota_i[:])
            ones_row = cst.tile([1, bb * 128], BF16, tag="ones_row")
            nc.gpsimd.memset(ones_row[:], 1.0)

            srcT_sb = cst.tile([128, T], I32, tag="idx_srcT")
            nc.sync.dma_start(srcT_sb[:], di["srcT"][:, :])
            colT_sb = cst.tile([128, T], F32, tag="idx_colT")
            nc.sync.dma_start(colT_sb[:], di["colT"][:, :])

            mats = {}
            for nm in ("A1m", "A2m", "B1m", "B2m"):
                mt = cst.tile([128, d], F32, tag=nm)
                nc.sync.dma_start(mt[:], di[nm][:, :])
                mats[nm] = mt
            w_sb = {}
            for nm in ("W1", "W2"):
                w_sb[nm] = []
                for (s, e) in dch:
                    wt = cst.tile([e - s, d + 1], BF16, tag=f"{nm}_{s}")
                    nc.sync.dma_start(wt[:], di[nm][s:e, :])
                    w_sb[nm].append(wt)

            ex_store = cst.tile([128, T], F32, tag="ex_store")
            rd1_store = cst.tile([128, nblk], F32, tag="rd1_store")
            sf1_store = cst.tile([128, nblk], F32, tag="sf1_store")
            v_sb = cst.tile([128, nblk * (d + 1)], BF16, tag="v_sb")
            e1T_hi = cst.tile([128, npad], BF16, tag="e1T_hi")
            e1T_lo = cst.tile([128, npad], BF16, tag="e1T_lo")
            e2T_hi = cst.tile([128, npad], BF16, tag="e2T_hi")
            e2T_lo = cst.tile([128, npad], BF16, tag="e2T_lo")

            # rotating per-tile buffers with constant regions hoisted
            uaug3 = []
            for k in range(3):
                ua = cst.tile([128, 2 * d + 2], BF16, tag=f"uaug{k}")
                nc.gpsimd.memset(ua[:, d + 1:d + 2], 1.0)
                uaug3.append(ua)
            ebf3 = []
            for k in range(3):
                eb = cst.tile([128, 256], BF16, tag=f"ebf{k}")
                nc.gpsimd.memset(eb[:, d:256], 0.0)
                ebf3.append(eb)

            # v1 from host-transposed ent slice
            entT_hi = cst.tile([128, npad], BF16, tag="entT_hi")
            nc.sync.dma_start(entT_hi[:], di["entT_hi"][:, :])
            entT_lo = cst.tile([d - 128, npad], BF16, tag="entT_lo")
            nc.sync.dma_start(entT_lo[:], di["entT_lo"][:, :])
            _emit_v_phase(nc, vpool, ps_u, entT_hi, entT_lo, di["Ws1"], v_sb,
                          nblk, d)

            # ---- layer 1 (AllGather chunks fire as their blocks finish)
            ag = {}
            cob = sched["chunk_of_blk"]
            for k in range(sched["nag"]):
                last_blk = int(np.max(np.nonzero(cob == k)[0]))
                t0 = int(sched["ag_tb"][k])
                t1 = t0 + ncores * int(sched["ag_rows"][k])
                ag[last_blk] = (ent1_full, k, t0, t1)
            _emit_edge_layer(nc, 1, cst, epool, ps_u, ps_vd, ps_acc, sched, di,
                             None, w_sb["W1"], mats["A1m"], mats["B1m"],
                             srcT_sb, colT_sb, iota_f32, v_sb, uaug3, ident_bf,
                             ebf3, ex_store, rd1_store, sf1_store, ent1_slk,
                             e1T_hi, e1T_lo, d, ag=ag)

            # v2 from resident ent1T (overlaps the AllGather)
            _emit_v_phase(nc, vpool, ps_u, e1T_hi, e1T_lo, di["Ws2"], v_sb,
                          nblk, d)

            # ---- layer 2
            _emit_edge_layer(nc, 2, cst, epool, ps_u, ps_vd, ps_acc, sched, di,
                             ent1_full, w_sb["W2"], mats["A2m"], mats["B2m"],
                             srcT_sb, colT_sb, iota_f32, v_sb, uaug3, ident_bf,
                             ebf3, ex_store, rd1_store, sf1_store, ent2_sl,
                             e2T_hi, e2T_lo, d)

            # ---- decoder
            _emit_decoder(nc, cst, vpool, ps_u, sched, di,
                          (ent2_sl, e2T_hi, e2T_lo, head_in, head_ar,
                           z_in, z_ar, scores_out, ident_bf, ones_row),
                          d, b, sched["prelu1"], sched["prelu2"])

    nc.compile()
    return nc


# ---------------------------------------------------------------- entry

_CACHE = {}


def _run(inputs, cfg, sim=False, trace=False):
    common, per_core, sched = _preprocess(inputs, cfg)
    key = (tuple(sorted(cfg.items())), sched["T"], tuple(sched["tpb"]))
    if key not in _CACHE:
        _CACHE[key] = build_program(common, per_core, sched, cfg)
    nc = _CACHE[key]
    in_maps = []
    for c in range(cfg["ncores"]):
        m = dict(common)
        m.update(per_core[c])
        in_maps.append({k: np.ascontiguousarray(v) for k, v in m.items()})
    if sim:
        from concourse.bass_interp import MultiCoreSim
        ms = MultiCoreSim(nc, num_cores=cfg["ncores"])
        for c in range(cfg["ncores"]):
            for name, arr in in_maps[c].items():
                ms.cores[c].tensor(name)[:] = arr
        ms.simulate(check_with_hw=False)
        outs = [np.array(ms.cores[c].tensor("scores")) for c in range(cfg["ncores"])]
        cat = np.concatenate(outs, axis=1)
        return np.ascontiguousarray(cat[:, sched["slot_of"]]), None
    res = bass_utils.run_bass_kernel_spmd(
        nc, in_maps, core_ids=list(range(cfg["ncores"])), trace=trace)
    outs = [res.results[c]["scores"] for c in range(cfg["ncores"])]
    cat = np.concatenate(outs, axis=1).astype(np.float32)
    return np.ascontiguousarray(cat[:, sched["slot_of"]]), res


def kernel(**inputs):
    out, _ = _run(inputs, FULL_CFG)
    return out
